# revision 6
# baseline (speedup 1.0000x reference)
"""Trainium2 Bass kernel for nn_CausalSelfAttention_57861799412149 (v4: head-sharded).

Sharding: core c = (batch b=c//4, kv-group g=c%4). Each core computes the 4
q-heads of one kv group over the FULL T=2048 sequence. Band-exact 128-block
coverage (108 blocks/head, zero padding), K/V projected exactly once per
(batch, group). Out-projection emits a bf16 PARTIAL product (its 4 heads
through the matching wproj rows); the host sums the 4 partials per batch.

Kernel tricks carried over from v3 + new:
- K^T pair-duplicated in SBUF via doubled stationary columns (free on PE).
- S^T via two 64-contraction matmuls at base partitions 0/64 (concurrent PE
  tiles); sliding-window masks folded in as (-60*I) x triangle matmuls.
- Softmax denominators from a ones-column in V; reciprocal via the fast
  custom-DVE approx, batched [4, 512] per query group.
- rmsnorm rsqrt via DVE Newton iteration + bc2r matmul broadcast.
- exp has no max-subtraction (|s| <= 8).
"""
import sys

sys.path.insert(0, "/opt/trn_rl_repo")

import numpy as np
import ml_dtypes

import concourse.bass as bass
import concourse.tile as tile
from concourse import bacc, mybir

B, T, NE = 2, 2048, 1024
NH, NKV, HD = 16, 4, 64
NTB = T // 128      # 16 token blocks
EK = NE // 128      # 8 contraction tiles
NQG = 4             # query groups of 512
WB = 8              # window in 128-blocks

f32 = mybir.dt.float32
f32r = mybir.dt.float32r
bf16 = mybir.dt.bfloat16
i32 = mybir.dt.int32
AF = mybir.ActivationFunctionType
OP = mybir.AluOpType
SWAP_MASK = [m for i in range(0, 32, 2) for m in (i + 1, i)]
MAGIC0 = 0x5F3759DF

_COMPILED = None


def _r(ap):
    return ap.bitcast(f32r)


def jt_range(qg):
    return range(4 * qg, min(4 * qg + 3 + WB, NTB - 1) + 1)


def build_program(repeat=1, unroll=False):
    nc = bacc.Bacc("TRN2", target_bir_lowering=False, debug=False, num_devices=8)

    def din(name, shape, dt=bf16):
        return nc.dram_tensor(name, shape, dt, kind="ExternalInput").ap()

    xt_d = din("xt", [NE, T])
    wq_d = din("wqt", [NE, 256])
    wkd_d = din("wkd", [NE, 128])
    wv_d = din("wvt", [NE, HD])
    wg_d = din("wgt", [32, 1])
    wp_d = din("wpt", [256, NE])
    csa_d = din("csa", [128, T])
    csbs_d = din("csbs", [128, T])
    ve_d = din("ve", [T, HD])
    exlo_d = din("exlo", [128, 128])
    exup_d = din("exup", [128, 128])
    negi_d = din("negi", [128, 128])
    bd_d = din("bdp", [128, 392], f32)
    bc2r_d = din("bc2r", [128, 128], f32)
    out_d = nc.dram_tensor("out", [T, NE], bf16, kind="ExternalOutput").ap()
    import os
    if os.environ.get("K2_DBG"):
        dbg_kt_d = nc.dram_tensor("dbg_kt", [128, T], bf16, kind="ExternalOutput").ap()
        dbg_qt_d = nc.dram_tensor("dbg_qt", [128, T], bf16, kind="ExternalOutput").ap()
        dbg_vx_d = nc.dram_tensor("dbg_vx", [128, HD + 1], bf16, kind="ExternalOutput").ap()
        dbg_yt_d = nc.dram_tensor("dbg_yt", [128, T], bf16, kind="ExternalOutput").ap()

    ctx_vars = locals()
    with tile.TileContext(nc) as tc:
        if repeat == 1:
            _build(nc, tc, ctx_vars)
        elif unroll:
            for _ in range(repeat):
                _build(nc, tc, ctx_vars)
        else:
            with tc.For_i(0, repeat,
                          hint_engines=(mybir.EngineType.PE,
                                        mybir.EngineType.DVE,
                                        mybir.EngineType.Activation)):
                _build(nc, tc, ctx_vars)

    nc.compile()
    return nc


def _build(nc, tc, d):
    from contextlib import ExitStack

    ctx = ExitStack()
    with ctx:
        # ---------------- persistent pools ----------------
        consts = ctx.enter_context(tc.tile_pool(name="consts", bufs=1))
        qtp = ctx.enter_context(tc.tile_pool(name="qtp", bufs=1))
        ktp = ctx.enter_context(tc.tile_pool(name="ktp", bufs=1))
        vxp = ctx.enter_context(tc.tile_pool(name="vxp", bufs=1))
        ytp = ctx.enter_context(tc.tile_pool(name="ytp", bufs=1))
        wqp = ctx.enter_context(tc.tile_pool(name="wqp", bufs=1))
        wpp = ctx.enter_context(tc.tile_pool(name="wpp", bufs=1))
        xa = ctx.enter_context(tc.tile_pool(name="xa", bufs=1))
        vep = ctx.enter_context(tc.tile_pool(name="vep", bufs=16))
        tmpA = ctx.enter_context(tc.tile_pool(name="tmpA", bufs=3))
        rotp = ctx.enter_context(tc.tile_pool(name="rotp", bufs=6))
        rsq = ctx.enter_context(tc.tile_pool(name="rsq", bufs=2))

        bdp = consts.tile([128, 392], f32r, tag="bdp")
        bc2r = consts.tile([128, 128], f32r, tag="bc2r")
        negi = consts.tile([128, 128], bf16, tag="negi")
        exlo = consts.tile([128, 128], bf16, tag="exlo")
        exup = consts.tile([128, 128], bf16, tag="exup")
        zP = consts.tile([128, 512], bf16, tag="zP")
        nc.vector.memset(zP[:], 0.0)
        csa = consts.tile([128, T], bf16, tag="csa")
        csbs = consts.tile([128, T], bf16, tag="csbs")

        qt = [qtp.tile([128, T], bf16, tag=f"qt{p}", name=f"qt{p}") for p in range(2)]
        kt = ktp.tile([128, T], bf16, tag="kt", name="kt")
        vx = [vxp.tile([128, HD + 1], bf16, tag=f"vx{j}", name=f"vx{j}")
              for j in range(NTB)]
        yt = [ytp.tile([128, T], bf16, tag=f"yt{f}", name=f"yt{f}") for f in range(2)]
        wqs = wqp.tile([128, 8 * 256], bf16, tag="wqs")
        wq = [wqs[:, 256 * e:256 * e + 256] for e in range(EK)]
        wps = wpp.tile([128, 2 * NE], bf16, tag="wps")
        wp = [wps[:, NE * f:NE * f + NE] for f in range(2)]

        wg = xa.tile([32, 1], bf16, tag="wg")
        nc.sync.dma_start(wg[:], d["wg_d"][:])
        xts = [xa.tile([128, 4 * T], bf16, tag=f"xts{a}", name=f"xts{a}")
               for a in range(2)]
        xt = [xts[e // 4][:, T * (e % 4):T * (e % 4) + T] for e in range(EK)]
        wkds = xa.tile([128, 8 * 128], bf16, tag="wkds")
        wkd = [wkds[:, 128 * e:128 * e + 128] for e in range(EK)]
        wvs = xa.tile([128, 8 * HD], bf16, tag="wvs")
        wv = [wvs[:, HD * e:HD * e + HD] for e in range(EK)]
        for a in range(2):
            nc.sync.dma_start(
                xts[a][:].rearrange("p (e t) -> p e t", t=T)[:, :, 0:1024],
                d["xt_d"][512 * a:512 * a + 512, :].rearrange(
                    "(e p) t -> p e t", p=128)[:, :, 0:1024])
        nc.sync.dma_start(
            wkds[:].rearrange("p (e c) -> p e c", c=128),
            d["wkd_d"][:].rearrange("(e p) c -> p e c", p=128))
        nc.sync.dma_start(csa[:], d["csa_d"][:])
        nc.sync.dma_start(csbs[:], d["csbs_d"][:])
        nc.sync.dma_start(bdp[:], _r(d["bd_d"][:]))
        nc.sync.dma_start(bc2r[:], _r(d["bc2r_d"][:]))
        for a in range(2):
            nc.sync.dma_start(
                xts[a][:].rearrange("p (e t) -> p e t", t=T)[:, :, 1024:T],
                d["xt_d"][512 * a:512 * a + 512, :].rearrange(
                    "(e p) t -> p e t", p=128)[:, :, 1024:T])
        nc.sync.dma_start(
            wqs[:].rearrange("p (e c) -> p e c", c=256),
            d["wq_d"][:].rearrange("(e p) c -> p e c", p=128))
        nc.sync.dma_start(
            wvs[:].rearrange("p (e c) -> p e c", c=HD),
            d["wv_d"][:].rearrange("(e p) c -> p e c", p=128))
        ves = vep.tile([128, NTB * HD], bf16, tag="ves")
        nc.sync.dma_start(
            ves[:].rearrange("p (j c) -> p j c", c=HD),
            d["ve_d"][:].rearrange("(j p) c -> p j c", p=128))
        vets = [ves[:, HD * j:HD * j + HD] for j in range(NTB)]
        nc.sync.dma_start(negi[:], d["negi_d"][:])
        nc.sync.dma_start(exlo[:], d["exlo_d"][:])
        nc.sync.dma_start(exup[:], d["exup_d"][:])
        nc.sync.dma_start(
            wps[:].rearrange("p (f c) -> p f c", c=NE),
            d["wp_d"][:].rearrange("(f p) c -> p f c", p=128))

        # ---- rope machinery: batched Newton rsqrt ----
        class Rope:
            def __init__(self, psKQ, psRQ, psB):
                self.psKQ, self.psRQ, self.psB = psKQ, psRQ, psB
                self.batch, self.pqb = [], None

            def newton_step(self, y, pq_, dst):
                nr = pq_.shape[0]
                t = rsq.tile([128, 512], f32, tag="nt", name="nt")
                nc.vector.tensor_mul(t[0:nr, :], y, y)
                nc.vector.tensor_mul(t[0:nr, :], t[0:nr, :], pq_)
                nc.vector.tensor_scalar(t[0:nr, :], t[0:nr, :], -0.5, 1.5, OP.mult, OP.add)
                nc.vector.tensor_mul(dst, y, t[0:nr, :])

            def flush(self):
                if not self.batch:
                    return
                pqb = self.pqb
                nr = 32 * (len(self.batch) - 1) + 2
                pq_ = pqb[0:nr, :]
                ii = rsq.tile([128, 512], i32, tag="ii", name="ii")
                nc.vector.tensor_scalar(ii[0:nr, :], pq_.bitcast(i32), 1, 0,
                                        OP.logical_shift_right)
                nc.vector.tensor_scalar(ii[0:nr, :], ii[0:nr, :], -1, MAGIC0,
                                        OP.mult, OP.add)
                y0 = ii[0:nr, :].bitcast(f32)
                rcp = rsq.tile([128, 512], f32r, tag="rcp", name="rcp")
                with nc.allow_low_precision(reason="rsqrt scale in f32r for matmul bcast"):
                    self.newton_step(y0, pq_, rcp[0:nr, :])
                for (s, rot, w, outs) in self.batch:
                    prq = self.psRQ.tile([128, w], f32, tag="prq", name="prq")
                    nc.tensor.matmul(prq[:], bc2r[32 * s:32 * s + 2, :],
                                     rcp[32 * s:32 * s + 2, 0:w],
                                     start=True, stop=True, tile_position=(32 * s, 0))
                    nc.vector.tensor_mul(outs, rot[:], prq[:])
                self.batch = []
                self.pqb = None

            BDP_OFF = [0, 98, 196, 294]

            def up(self, pr, c0, w, outs, nb=4, last=False):
                if self.pqb is None:
                    self.pqb = self.psB.tile([128, 512], f32, tag="pqb", name="pqb")
                s = len(self.batch)
                rot = rotp.tile([128, w], bf16, tag="rot", name="rot")
                prC = tmpA.tile([128, w], bf16, tag="prc", name="prc")
                nc.scalar.copy(prC[:], pr[:])
                prS = tmpA.tile([128, w], bf16, tag="prs", name="prs")
                nc.vector.stream_shuffle(prS[:], prC[:], SWAP_MASK)
                ta = tmpA.tile([128, w], bf16, tag="ta", name="ta")
                nc.vector.tensor_mul(ta[:], prC[:], csa[:, c0:c0 + w])
                tbs = tmpA.tile([128, w], bf16, tag="tbs", name="tbs")
                nc.gpsimd.tensor_mul(tbs[:], prS[:], csbs[:, c0:c0 + w])
                # alternate the add between DVE and Pool to balance phase A
                if len(self.batch) % 2 == 0:
                    nc.vector.tensor_add(rot[:], ta[:], tbs[:])
                else:
                    nc.gpsimd.tensor_add(rot[:], ta[:], tbs[:])
                sq = tmpA.tile([128, w], f32r, tag="sq", name="sq")
                nc.scalar.activation(sq[:], rot[:], AF.Square)
                off = self.BDP_OFF[s]
                nr = 32 * (nb - 1) + 2
                nc.tensor.matmul(self.pqb[0:nr, 0:w],
                                 bdp[:, off:off + nr], sq[:],
                                 start=(s == 0), stop=last)
                self.batch.append((s, rot, w, outs))
                if last:
                    self.flush()

        # ========== phase A ==========
        with (
            tc.tile_pool(name="psPR", bufs=2, space="PSUM") as psPR,
            tc.tile_pool(name="psRQ", bufs=2, space="PSUM") as psRQ,
            tc.tile_pool(name="psV", bufs=2, space="PSUM") as psV,
            tc.tile_pool(name="psB", bufs=2, space="PSUM") as psB,
        ):
            ropeA = Rope(psPR, psRQ, psB)

            def kchunk(c):
                c0 = 512 * c
                pr = psPR.tile([128, 512], f32, tag="pk", name="pk")
                for e in range(EK):
                    nc.tensor.matmul(pr[:], wkd[e][:],
                                     xt[e][:, c0:c0 + 512],
                                     start=(e == 0), stop=(e == EK - 1))
                ropeA.up(pr, c0, 512, kt[:, c0:c0 + 512], nb=4, last=(c == 3))

            # K chunks 0-1 need only the first token half of x
            kchunk(0)
            kchunk(1)

            # gates: z_j = x[:, :32] @ wg  per key block -> pg[:, j]
            pg = psV.tile([128, HD], f32, tag="pv", name="pg")
            for j in range(NTB):
                nc.tensor.matmul(pg[:, j:j + 1], xt[0][0:32, 128 * j:128 * j + 128],
                                 wg[:], start=True, stop=True)
            gt = xa.tile([128, NTB], f32, tag="gt")
            nc.scalar.activation(gt[:], pg[:, 0:NTB], AF.Tanh, scale=0.5)
            g2 = xa.tile([128, NTB], bf16, tag="g2")
            nc.vector.tensor_scalar_add(g2[:], gt[:], 1.0)

            kchunk(2)
            kchunk(3)

            # Q (2 pairs) + rope, chunk-major so attention can start early
            for c in range(4):
                c0 = 512 * c
                for p in range(2):
                    pr = psPR.tile([128, 512], f32, tag="pk", name="pq")
                    for e in range(EK):
                        nc.tensor.matmul(pr[:], wq[e][:, 128 * p:128 * p + 128],
                                         xt[e][:, c0:c0 + 512],
                                         start=(e == 0), stop=(e == EK - 1))
                    ropeA.up(pr, c0, 512, qt[p][:, c0:c0 + 512], nb=4,
                             last=(c % 2 == 1 and p == 1))

            # V per key block + ve gate + ones column
            for j in range(NTB):
                pv = psV.tile([128, HD], f32, tag="pv", name="pv")
                for e in range(EK):
                    nc.tensor.matmul(pv[:], xt[e][:, 128 * j:128 * j + 128],
                                     wv[e][:], start=(e == 0), stop=(e == EK - 1))
                vg = vep.tile([128, HD], bf16, tag="vg", name="vg", bufs=2)
                nc.gpsimd.tensor_mul(
                    vg[:], vets[j][:],
                    g2[:, j:j + 1].broadcast_to([128, HD]))
                nc.vector.tensor_add(vx[j][:, 0:HD], vg[:], pv[:])
                nc.vector.memset(vx[j][:, HD:HD + 1], 1.0)

        # ========== phase B: attention + interleaved out projection ==========
        with (
            tc.tile_pool(name="tmpB", bufs=2) as tmpB,
            tc.tile_pool(name="ptp", bufs=4) as ptp,
            tc.tile_pool(name="psS", bufs=2, space="PSUM") as psS,
            tc.tile_pool(name="psO", bufs=4, space="PSUM") as psO,
        ):
            for qg in range(NQG):
                ots = {}
                for p in range(2):
                    for idx in range(2):
                        ot = psO.tile([HD + 1, 512], f32, tag="ot",
                                      name=f"ot{qg}_{p}_{idx}")
                        nc.tensor.matmul(ot[:], vx[4 * qg][:], zP[:],
                                         start=True, stop=False)
                        ots[(p, idx)] = ot
                for p in range(2):
                    for jt in jt_range(qg):
                        il0 = max(4 * qg, jt - WB)
                        il1 = min(jt, 4 * qg + 3)
                        iw0 = 128 * (il0 - 4 * qg)
                        w = 128 * (il1 - il0 + 1)
                        diag = jt <= 4 * qg + 3
                        bup = jt - WB >= 4 * qg
                        q0 = 512 * qg + iw0
                        s2 = psS.tile([128, 1024], f32, tag="st", name="st")
                        nc.tensor.matmul(s2[:, 0:w],
                                         kt[0:64, 128 * jt:128 * jt + 128],
                                         qt[p][0:64, q0:q0 + w],
                                         start=True, stop=not (diag or bup))
                        nc.tensor.matmul(s2[:, 512:512 + w],
                                         kt[64:128, 128 * jt:128 * jt + 128],
                                         qt[p][64:128, q0:q0 + w],
                                         start=True, stop=not (diag or bup))
                        if diag:
                            nc.tensor.matmul(s2[:, w - 128:w], negi[:], exlo[:],
                                             start=False, stop=True)
                            nc.tensor.matmul(s2[:, 512 + w - 128:512 + w],
                                             negi[:], exlo[:],
                                             start=False, stop=True)
                        if bup:
                            nc.tensor.matmul(s2[:, 0:128], negi[:], exup[:],
                                             start=False, stop=True)
                            nc.tensor.matmul(s2[:, 512:512 + 128],
                                             negi[:], exup[:],
                                             start=False, stop=True)
                        pt = ptp.tile([128, 1024], bf16, tag="pt", name="pt")
                        sv = s2[:].rearrange("q (b c) -> q b c", b=2)[:, :, 0:w]
                        pv_ = pt[:].rearrange("q (b c) -> q b c", b=2)[:, :, 0:w]
                        nc.scalar.activation(pv_, sv, AF.Exp)
                        last = (jt == list(jt_range(qg))[-1])
                        for idx in range(2):
                            off = 512 * idx
                            nc.tensor.matmul(ots[(p, idx)][:, iw0:iw0 + w],
                                             vx[jt][:], pt[:, off:off + w],
                                             start=False, stop=last)
                # normalize the 4 heads of this query group
                for p in range(2):
                    for idx in range(2):
                        h = 2 * p + idx
                        ds = tmpB.tile([1, 512], f32, tag="ds", name=f"ds{qg}_{h}")
                        nc.vector.tensor_copy(ds[:], ots[(p, idx)][HD:HD + 1, :])
                        rs = tmpB.tile([1, 512], f32, tag="rs", name=f"rs{qg}_{h}")
                        nc.vector.reciprocal_approx_fast(rs[:], ds[:])
                        rsb = tmpB.tile([64, 512], f32, tag="rsb", name=f"rsb{h}")
                        nc.gpsimd.partition_broadcast(rsb[:], rs[:])
                        nc.vector.tensor_mul(
                            yt[p][64 * idx:64 * idx + 64,
                                  512 * qg:512 * qg + 512],
                            ots[(p, idx)][0:HD, :], rsb[:])

        import os
        if os.environ.get("K2_DBG"):
            nc.sync.dma_start(d["dbg_kt_d"][:], kt[:])
            nc.sync.dma_start(d["dbg_qt_d"][:], qt[0][:])
            nc.sync.dma_start(d["dbg_vx_d"][:], vx[0][:])
            nc.sync.dma_start(d["dbg_yt_d"][:], yt[0][:])

        # ========== phase C: partial out projection ==========
        with (
            tc.tile_pool(name="pop", bufs=2) as pop2,
            tc.tile_pool(name="psP", bufs=4, space="PSUM") as psP,
        ):
            for a in range(4):
                po = pop2.tile([128, 4 * NE], bf16, tag="po", name="po")
                for tb4 in range(4):
                    tb = 4 * a + tb4
                    for half in range(2):
                        pp = psP.tile([128, 512], f32, tag="pp", name="pp")
                        for f in range(2):
                            nc.tensor.matmul(pp[:], yt[f][:, 128 * tb:128 * tb + 128],
                                             wp[f][:, 512 * half:512 * half + 512],
                                             start=(f == 0), stop=(f == 1))
                        dst = po[:, NE * tb4 + 512 * half:NE * tb4 + 512 * half + 512]
                        if half == 0:
                            nc.scalar.copy(dst, pp[:])
                        else:
                            nc.vector.tensor_copy(dst, pp[:])
                nc.sync.dma_start(
                    d["out_d"][512 * a:512 * a + 512, :].rearrange(
                        "(tb p) o -> p tb o", p=128),
                    po[:].rearrange("p (tb o) -> p tb o", o=NE))


# ---------------- host prep ----------------

def host_prep(inputs):
    bfd = ml_dtypes.bfloat16
    x = np.asarray(inputs["x"], np.float32)
    ve = np.asarray(inputs["ve"], np.float32)
    cos = np.asarray(inputs["cos"], np.float32)
    sin = np.asarray(inputs["sin"], np.float32)
    wq = np.asarray(inputs["wq"], np.float32)
    wk = np.asarray(inputs["wk"], np.float32)
    wv = np.asarray(inputs["wv"], np.float32)
    wproj = np.asarray(inputs["wproj"], np.float32)
    wgate = np.asarray(inputs["wgate"], np.float32)

    def rope_perm(nh):
        idx = np.empty(nh * 64, np.int64)
        for h in range(nh):
            for dd in range(32):
                for half in range(2):
                    idx[h * 64 + 2 * dd + half] = h * 64 + 32 * half + dd
        return idx

    XT = np.ascontiguousarray(x.transpose(0, 2, 1)).astype(bfd)       # [B, NE, T]
    wq_perm = wq.T[:, rope_perm(NH)].astype(bfd)                       # [NE, 1024]
    wk_perm = wk.T[:, rope_perm(NKV)].astype(bfd)                      # [NE, 256]
    wv_t = wv.T.astype(bfd)                                            # [NE, 256]
    wp_t = wproj.T.astype(bfd)                                         # [NE(in), NE(out)]
    wg_t = wgate.T.astype(bfd)                                         # [32, NKV]

    cosT = cos[0, :, 0, :].T.astype(np.float32)                        # [32, T]
    sinT = sin[0, :, 0, :].T.astype(np.float32)
    csa64 = np.empty((64, T), np.float32)
    csb64 = np.empty((64, T), np.float32)
    csa64[0::2] = cosT
    csa64[1::2] = cosT
    csb64[0::2] = -sinT
    csb64[1::2] = sinT
    CSA = np.concatenate([csa64, csa64], 0)
    CSB = np.concatenate([csb64, csb64], 0)
    perm = np.arange(128) ^ 1
    CSBS = CSB[perm]

    jj = np.arange(128)[:, None]   # key (partition)
    ii = np.arange(128)[None, :]   # query (column)
    exlo = (ii > jj).astype(bfd)
    exup = (ii < jj).astype(bfd)
    negi = (-60.0 * np.eye(128)).astype(bfd)
    bdp = np.zeros((128, 392), np.float32)
    for s in range(4):
        bdp[:64, 98 * s + 32 * s] = 0.125
        bdp[64:, 98 * s + 32 * s + 1] = 0.125
    bc2r = np.zeros((128, 128), np.float32)
    for s in range(4):
        bc2r[32 * s, :64] = 1.0
        bc2r[32 * s + 1, 64:] = 1.0

    CSAb = CSA.astype(bfd)
    CSBSb = CSBS.astype(bfd)

    in_maps = []
    for c in range(8):
        b, g = c // 4, c % 4
        wkg = wk_perm[:, 64 * g:64 * g + 64]
        in_maps.append({
            "xt": XT[b],
            "wqt": np.ascontiguousarray(wq_perm[:, 256 * g:256 * g + 256]),
            "wkd": np.ascontiguousarray(np.concatenate([wkg, wkg], axis=1)),
            "wvt": np.ascontiguousarray(wv_t[:, 64 * g:64 * g + 64]),
            "wgt": np.ascontiguousarray(wg_t[:, g:g + 1]),
            "wpt": np.ascontiguousarray(wp_t[256 * g:256 * g + 256, :]),
            "csa": CSAb, "csbs": CSBSb,
            "ve": np.ascontiguousarray(ve[b][:, 64 * g:64 * g + 64].astype(bfd)),
            "exlo": exlo, "exup": exup, "negi": negi,
            "bdp": bdp, "bc2r": bc2r,
        })
    return in_maps


def kernel(**inputs):
    global _COMPILED
    if _COMPILED is None:
        _COMPILED = build_program()
    nc = _COMPILED
    in_maps = host_prep(inputs)

    from concourse.bass_utils import run_bass_kernel_spmd
    res = run_bass_kernel_spmd(nc, in_maps, list(range(8)))

    out = np.empty((B, T, NE), np.float32)
    for b in range(B):
        acc = np.zeros((T, NE), np.float32)
        for g in range(4):
            acc += np.asarray(res.results[4 * b + g]["out"]).astype(np.float32)
        out[b] = acc
    return out
